# revision 11
# baseline (speedup 1.0000x reference)
"""Trainium2 Bass kernel for nn_DecoderLayerJ (GNN message-passing decoder layer).

Strategy: data-parallel over the 8 NeuronCores — each core owns 1/8 of the
B*N nodes (1024 nodes) plus all weights (replicated). Inside a core the
pipeline runs feature-major ([128 feature partitions x node/edge columns]):

  h_e (fp32, 24.6MB/core) --SWDGE cast-dma--> fp16 natural layout
      --HWDGE xbar transpose--> h_eT [d, edges]
  z1 = W1e@h_eT + W1v@h_vT(col-broadcast rhs)      (PSUM accumulate)
  m1 = gelu(z1 + b1)                               (ACT, bias fused, fp16 out)
  z2 = W2@m1 + ones x ((mask-1)*1e4)               (rank-1 mask bias: binary
                                                    mask => gelu(z-1e4) == 0)
  m2m = gelu(z2 + b2)  == mask * gelu(W2 m1 + b2)  - correction not needed
  s2 = sum_k m2m                                   (DVE strided reduce)
  dh = (W3@s2 + b3 x msum) / 30                    (K-sum commutes past W3)
  LN1/LN2 feature-major: column sums via ones-matmul, rsqrt via Newton on
  DVE (no ACT table switches), per-node coeffs broadcast via rank-1 matmuls,
  mask_v folded into the LN2 coefficients.

Output is produced feature-major [128, 1024] per core and re-assembled /
transposed on the host during the unshard step.
"""

import os
import sys
from contextlib import ExitStack

os.environ.setdefault("MYCRO_LOCAL_CACHE", "1")
for _p in ("/opt/trn_rl_repo", "/root/.axon_site/_ro/trn_rl_repo"):
    if os.path.isdir(_p) and _p not in sys.path:
        sys.path.append(_p)

import numpy as np  # noqa: E402

import concourse.bacc as bacc  # noqa: E402
import concourse.bass as bass  # noqa: E402
import concourse.tile as tile  # noqa: E402
from concourse import mybir  # noqa: E402
from concourse.bass_utils import run_bass_kernel_spmd  # noqa: E402
from concourse.masks import make_identity  # noqa: E402

F32 = mybir.dt.float32
F16 = mybir.dt.float16
AX = mybir.AxisListType
ALU = mybir.AluOpType
ACTF = mybir.ActivationFunctionType

N_CORES = 8
B, N, K, H, IN = 4, 2048, 48, 128, 128
H4 = 4 * H
SCALE = 30.0
EPS = 1e-5
BIG = 1.0e4

TPT = 8            # nodes per tile -> 384 edge columns, bank-aligned at 512
RG = 8             # tiles per reduce group (3072 edge columns)


def _emit(tc: "tile.TileContext", tin: dict, tout: dict, nodes: int):
    nc = tc.nc
    NT = nodes // TPT          # tiles (<= 128)
    NRG = NT // RG             # reduce groups
    ECOL = RG * TPT * K        # 3072 edge cols per reduce group
    NBK = ECOL // 128          # 24 transpose blocks per reduce group
    NB = nodes // 128          # gathered width (8)
    CH = min(512, nodes)       # dense-phase node chunk
    NCH = nodes // CH
    assert NT <= 128 and NT % RG == 0 and nodes % 128 == 0

    ctx = ExitStack()
    with ctx:
        consts = ctx.enter_context(tc.tile_pool(name="consts", bufs=1))
        big = ctx.enter_context(tc.tile_pool(name="big", bufs=1))
        work = ctx.enter_context(tc.tile_pool(name="work", bufs=2))

        # ---- constants / weights ----
        def cload(name, shape, dt, in_ap=None):
            t = consts.tile(shape, dt, tag=f"c_{name}")
            nc.sync.dma_start(out=t, in_=tin[name] if in_ap is None else in_ap)
            return t

        w1eT = cload("w1eT", [IN, H], F16)
        w1vT = cload("w1vT", [H, H], F32)
        w2T = cload("w2T", [H, H], F16)
        w3T = cload("w3T", [H, H], F32)
        d1T = cload("d1T", [H, H4], F32)
        d2Tq = cload("d2T", [128, 4, H], F16,
                     in_ap=tin["d2T"].rearrange("(q p) h -> p q h", p=128))
        b1t = cload("b1", [H, 1], F32)
        b2t = cload("b2", [H, 1], F32)
        db1q = cload("db1", [128, 4], F32,
                     in_ap=tin["db1"].rearrange("(q p) one -> p (q one)", p=128))
        b3row = cload("b3row", [1, H], F32)
        db2row = cload("db2row", [1, H], F32)
        g1row = cload("g1row", [1, H], F32)
        beta1row = cload("beta1row", [1, H], F32)
        g2row = cload("g2row", [1, H], F32)
        beta2row = cload("beta2row", [1, H], F32)
        mvg = cload("mask_v", [128, NB], F32)

        g1neg = consts.tile([1, H], F32)
        nc.vector.tensor_scalar_mul(g1neg, g1row, -1.0)
        g2neg = consts.tile([1, H], F32)
        nc.vector.tensor_scalar_mul(g2neg, g2row, -1.0)

        ones_col = consts.tile([H, 1], F32)
        nc.vector.memset(ones_col, 1.0)
        ones_r1 = consts.tile([1, H], F32)      # lhsT for rank-1 column bias
        nc.vector.memset(ones_r1, 1.0)
        ones_row = consts.tile([1, CH], F32)
        nc.vector.memset(ones_row, 1.0)

        ident = consts.tile([128, 128], F32)
        make_identity(nc, ident)

        # ---- mask prep ----
        mraw = consts.tile([NT, TPT * K], F32)
        nc.sync.dma_start(out=mraw, in_=tin["mask_attend"])
        msum = consts.tile([NT, TPT], F32)
        nc.vector.tensor_reduce(out=msum, in_=mraw.rearrange("p (i k) -> p i k", k=K),
                                axis=AX.X, op=ALU.add)
        cmask = consts.tile([NT, TPT * K], F32)
        nc.vector.tensor_scalar(cmask, mraw, BIG, -BIG, op0=ALU.mult, op1=ALU.add)
        msum_row = consts.tile([1, nodes], F32)
        nc.gpsimd.dma_start(out=msum_row, in_=msum)

        # ---- staging rows for LN stats gather/scatter ----
        stage = consts.tile([1, 2 * nodes], F32)    # [mu | msq] rows
        rows1 = consts.tile([1, 2 * nodes], F32)    # [rstd | mu*rstd] LN1
        rows2 = consts.tile([1, 3 * nodes], F32)    # [rstd*mv | mu*rstd*mv | mv] LN2

        hvT = big.tile([H, nodes], F32)
        dh = big.tile([H, nodes], F32)
        x = big.tile([H, nodes], F32)
        h1 = big.tile([H, nodes], F32)
        zbuf = big.tile([H, nodes], F32)

        with tc.tile_pool(name="pz1", bufs=2, space="PSUM") as pz1, \
             tc.tile_pool(name="pz2", bufs=1, space="PSUM") as pz2, \
             tc.tile_pool(name="psm", bufs=2, space="PSUM") as psm, \
             tc.tile_pool(name="phe", bufs=2) as phe, \
             tc.tile_pool(name="phet", bufs=2) as phet, \
             tc.tile_pool(name="pm1", bufs=2) as pm1, \
             tc.tile_pool(name="pm2m", bufs=2) as pm2m, \
             tc.tile_pool(name="ps2", bufs=2) as ps2, \
             tc.tile_pool(name="pcr", bufs=2) as pcr:

            # h_v transpose: [nodes, H] -> hvT [H, nodes] via PE
            for b in range(nodes // 128):
                hv_nat = work.tile([128, H], F32, tag="hvnat")
                nc.sync.dma_start(out=hv_nat, in_=tin["h_v"][b * 128:(b + 1) * 128, :])
                ps = psm.tile([128, 128], F32, tag="s")
                nc.tensor.transpose(ps, hv_nat, ident)
                nc.vector.tensor_copy(hvT[:, b * 128:(b + 1) * 128], ps)

            # ---- edge phase ----
            for rg in range(NRG):
                r0 = rg * ECOL
                henat = phe.tile([128, ECOL], F16)
                nc.gpsimd.dma_start(
                    out=henat.rearrange("p (b d) -> p b d", d=128),
                    in_=tin["h_e"][r0:r0 + ECOL, :].rearrange("(b p) d -> p b d", p=128),
                )
                crg = pcr.tile([1, ECOL], F32)
                nc.sync.dma_start(out=crg, in_=cmask[rg * RG:(rg + 1) * RG, :])
                heT = phet.tile([128, ECOL], F16)
                for b in range(NBK):
                    nc.sync.dma_start_transpose(
                        out=heT[:, b * 128:(b + 1) * 128],
                        in_=henat[:, b * 128:(b + 1) * 128])

                m2m = pm2m.tile([128, ECOL], F32)
                for g2 in range(RG // 2):
                    z1 = pz1.tile([128, 1024], F32)
                    for j in range(2):
                        t = rg * RG + g2 * 2 + j
                        ec = (g2 * 2 + j) * TPT * K
                        pc = j * 512
                        nc.tensor.matmul(z1[:, pc:pc + 384], lhsT=w1eT,
                                         rhs=heT[:, ec:ec + 384],
                                         start=True, stop=False)
                        hv_ap = hvT[:, t * TPT:(t + 1) * TPT]
                        rhs_hv = bass.AP(tensor=hv_ap.tensor, offset=hv_ap.offset,
                                         ap=[list(hv_ap.ap[0]), list(hv_ap.ap[1]), [0, K]])
                        nc.tensor.matmul(z1[:, pc:pc + 384], lhsT=w1vT, rhs=rhs_hv,
                                         start=False, stop=True)
                    m1 = pm1.tile([128, 2, 384], F16)
                    nc.scalar.activation(
                        out=m1,
                        in_=z1.rearrange("p (a b) -> p a b", b=512)[:, :, 0:384],
                        func=ACTF.Gelu, bias=b1t)
                    z2 = pz2.tile([128, 1024], F32)
                    for j in range(2):
                        t = rg * RG + g2 * 2 + j
                        pc = j * 512
                        nc.tensor.matmul(z2[:, pc:pc + 384], lhsT=w2T,
                                         rhs=m1[:, j, :], start=True, stop=False)
                        jj = g2 * 2 + j
                        nc.tensor.matmul(z2[:, pc:pc + 384], lhsT=ones_r1,
                                         rhs=crg[:, jj * 384:(jj + 1) * 384],
                                         start=False, stop=True)
                    nc.scalar.activation(
                        out=m2m[:, g2 * 768:(g2 + 1) * 768].rearrange(
                            "p (a b) -> p a b", b=384),
                        in_=z2.rearrange("p (a b) -> p a b", b=512)[:, :, 0:384],
                        func=ACTF.Gelu, bias=b2t)

                s2 = ps2.tile([128, RG * TPT], F32)
                nc.vector.tensor_reduce(out=s2,
                                        in_=m2m.rearrange("p (n k) -> p n k", k=K),
                                        axis=AX.X, op=ALU.add)
                dps = psm.tile([128, RG * TPT], F32, tag="s")
                nc.tensor.matmul(dps, lhsT=w3T, rhs=s2, start=True, stop=False)
                nc.tensor.matmul(dps, lhsT=b3row,
                                 rhs=msum_row[:, rg * RG * TPT:(rg + 1) * RG * TPT],
                                 start=False, stop=True)
                nc.scalar.mul(out=dh[:, rg * RG * TPT:(rg + 1) * RG * TPT],
                              in_=dps, mul=1.0 / SCALE)

        # ---- dense phase ----
        def ln_rows(src, stage_t, out_rows, with_mv):
            """Per-node LN coefficient rows from feature-major src [H, nodes].

            Writes sums into stage_t ([mu|msq]), gathers to [128, 2*NB],
            Newton-iterates rstd on DVE, scatters coefficient rows."""
            for ch in range(NCH):
                s = ch * CH
                srow = prow.tile([1, CH], F32)
                nc.tensor.matmul(srow, lhsT=ones_col, rhs=src[:, s:s + CH],
                                 start=True, stop=True)
                sq = pdense.tile([128, CH], F32, tag="d")
                nc.vector.tensor_mul(sq, src[:, s:s + CH], src[:, s:s + CH])
                qrow = prow.tile([1, CH], F32)
                nc.tensor.matmul(qrow, lhsT=ones_col, rhs=sq, start=True, stop=True)
                nc.vector.tensor_scalar_mul(stage_t[:, s:s + CH], srow, 1.0 / H)
                nc.vector.tensor_scalar_mul(stage_t[:, nodes + s:nodes + s + CH],
                                            qrow, 1.0 / H)
            g = pw.tile([128, 2, NB], F32, tag="g")
            for hh in range(2):
                sl = stage_t[:, hh * nodes:(hh + 1) * nodes]
                nc.sync.dma_start(
                    out=g[:, hh, :],
                    in_=bass.AP(tensor=sl.tensor, offset=sl.offset,
                                ap=[list(sl.ap[0]), [NB, 128], [1, NB]]))
            mug = g[:, 0, :]
            msqg = g[:, 1, :]
            tvar = pw.tile([128, NB], F32, tag="w")
            nc.vector.tensor_mul(tvar, mug, mug)
            tvar2 = pw.tile([128, NB], F32, tag="w")
            nc.vector.tensor_sub(tvar2, msqg, tvar)
            teps = pw.tile([128, NB], F32, tag="w")
            nc.vector.tensor_scalar_add(teps, tvar2, EPS)
            y = pw.tile([128, NB], F32, tag="w")
            nc.vector.reciprocal(y, teps)
            nc.vector.tensor_scalar_min(y, y, 1.7)
            for _ in range(5):
                yy = pw.tile([128, NB], F32, tag="w")
                nc.vector.tensor_mul(yy, y, y)
                nc.vector.tensor_mul(yy, yy, teps)
                nc.vector.tensor_scalar(yy, yy, -0.5, 1.5, op0=ALU.mult, op1=ALU.add)
                nc.vector.tensor_mul(y, y, yy)
            nhalf = 3 if with_mv else 2
            stg = pw.tile([128, nhalf, NB], F32, tag="g")
            if with_mv:
                nc.vector.tensor_mul(stg[:, 0, :], y, mvg)       # rstd*mv
                nc.vector.tensor_mul(stg[:, 1, :], mug, stg[:, 0, :])  # mu*rstd*mv
                nc.vector.tensor_copy(stg[:, 2, :], mvg)
            else:
                nc.vector.tensor_copy(stg[:, 0, :], y)
                nc.vector.tensor_mul(stg[:, 1, :], mug, y)
            for hh in range(nhalf):
                sl = out_rows[:, hh * nodes:(hh + 1) * nodes]
                nc.sync.dma_start(
                    out=bass.AP(tensor=sl.tensor, offset=sl.offset,
                                ap=[list(sl.ap[0]), [NB, 128], [1, NB]]),
                    in_=stg[:, hh, :])

        with tc.tile_pool(name="pu", bufs=2, space="PSUM") as pu, \
             tc.tile_pool(name="pab", bufs=1, space="PSUM") as pab, \
             tc.tile_pool(name="pv", bufs=1, space="PSUM") as pv, \
             tc.tile_pool(name="prow", bufs=1, space="PSUM") as prow, \
             tc.tile_pool(name="pdense", bufs=3) as pdense, \
             tc.tile_pool(name="pus", bufs=4) as pus, \
             tc.tile_pool(name="pw", bufs=8) as pw:

            nc.vector.tensor_add(x, hvT, dh)
            ln_rows(x, stage, rows1, with_mv=False)
            for ch in range(NCH):
                s = ch * CH
                A = pab.tile([128, CH], F32)
                nc.tensor.matmul(A, lhsT=g1row, rhs=rows1[:, s:s + CH],
                                 start=True, stop=True)
                Bt = pab.tile([128, CH], F32)
                nc.tensor.matmul(Bt, lhsT=beta1row, rhs=ones_row, start=True,
                                 stop=False)
                nc.tensor.matmul(Bt, lhsT=g1neg, rhs=rows1[:, nodes + s:nodes + s + CH],
                                 start=False, stop=True)
                tt = pdense.tile([128, CH], F32, tag="d")
                nc.vector.tensor_mul(tt, x[:, s:s + CH], A)
                nc.vector.tensor_add(h1[:, s:s + CH], tt, Bt)

                vps = pv.tile([128, CH], F32)
                for q in range(4):
                    ups = pu.tile([128, CH], F32)
                    nc.tensor.matmul(ups, lhsT=d1T[:, q * 128:(q + 1) * 128],
                                     rhs=h1[:, s:s + CH], start=True, stop=True)
                    uq = pus.tile([128, CH], F16)
                    nc.scalar.activation(out=uq, in_=ups, func=ACTF.Gelu,
                                         bias=db1q[:, q:q + 1])
                    nc.tensor.matmul(vps, lhsT=d2Tq[:, q, :], rhs=uq,
                                     start=(q == 0), stop=False)
                nc.tensor.matmul(vps, lhsT=db2row, rhs=ones_row, start=False,
                                 stop=True)
                nc.vector.tensor_add(zbuf[:, s:s + CH], h1[:, s:s + CH], vps)

            ln_rows(zbuf, stage, rows2, with_mv=True)
            for ch in range(NCH):
                s = ch * CH
                A = pab.tile([128, CH], F32)
                nc.tensor.matmul(A, lhsT=g2row, rhs=rows2[:, s:s + CH],
                                 start=True, stop=True)
                Bt = pab.tile([128, CH], F32)
                nc.tensor.matmul(Bt, lhsT=beta2row,
                                 rhs=rows2[:, 2 * nodes + s:2 * nodes + s + CH],
                                 start=True, stop=False)
                nc.tensor.matmul(Bt, lhsT=g2neg, rhs=rows2[:, nodes + s:nodes + s + CH],
                                 start=False, stop=True)
                tt = pdense.tile([128, CH], F32, tag="d")
                nc.vector.tensor_mul(tt, zbuf[:, s:s + CH], A)
                ot = pdense.tile([128, CH], F32, tag="d")
                nc.vector.tensor_add(ot, tt, Bt)
                nc.sync.dma_start(out=tout["out"][:, s:s + CH], in_=ot)


def build_bass(nodes: int):
    nc = bacc.Bacc("TRN2", target_bir_lowering=False, debug=False)
    tin = {}
    tin["h_e"] = nc.dram_tensor("h_e", [nodes * K, IN], F32, kind="ExternalInput").ap()
    tin["h_v"] = nc.dram_tensor("h_v", [nodes, H], F32, kind="ExternalInput").ap()
    tin["mask_attend"] = nc.dram_tensor(
        "mask_attend", [nodes // TPT, TPT * K], F32, kind="ExternalInput").ap()
    tin["mask_v"] = nc.dram_tensor(
        "mask_v", [128, nodes // 128], F32, kind="ExternalInput").ap()
    for name, shape, dt in [
        ("w1eT", [IN, H], F16), ("w1vT", [H, H], F32), ("w2T", [H, H], F16),
        ("w3T", [H, H], F32), ("d1T", [H, H4], F32), ("d2T", [H4, H], F16),
        ("b1", [H, 1], F32), ("b2", [H, 1], F32), ("db1", [H4, 1], F32),
        ("b3row", [1, H], F32), ("db2row", [1, H], F32),
        ("g1row", [1, H], F32), ("beta1row", [1, H], F32),
        ("g2row", [1, H], F32), ("beta2row", [1, H], F32),
    ]:
        tin[name] = nc.dram_tensor(name, shape, dt, kind="ExternalInput").ap()
    tout = {"out": nc.dram_tensor("out", [H, nodes], F32, kind="ExternalOutput").ap()}

    with tile.TileContext(nc) as tc:
        _emit(tc, tin, tout, nodes)
    nc.compile()
    return nc


def make_in_maps(inputs: dict, nodes_per_core: int, n_cores: int):
    """Shard activations over cores; replicate (pre-transposed) weights."""
    f32 = np.float32
    he = np.ascontiguousarray(inputs["h_e"], dtype=f32).reshape(B * N * K, IN)
    hv = np.ascontiguousarray(inputs["h_v"], dtype=f32).reshape(B * N, H)
    ma = np.ascontiguousarray(inputs["mask_attend"], dtype=f32).reshape(B * N, K)
    mv = np.ascontiguousarray(inputs["mask_v"], dtype=f32).reshape(B * N)
    W1, W2, W3 = inputs["W1"], inputs["W2"], inputs["W3"]
    D1, D2 = inputs["D1"], inputs["D2"]
    shared = {
        "w1eT": np.ascontiguousarray(np.asarray(W1, f32)[:, IN:].T, np.float16),
        "w1vT": np.ascontiguousarray(np.asarray(W1, f32)[:, :IN].T, f32),
        "w2T": np.ascontiguousarray(np.asarray(W2, f32).T, np.float16),
        "w3T": np.ascontiguousarray(np.asarray(W3, f32).T, f32),
        "d1T": np.ascontiguousarray(np.asarray(D1, f32).T, f32),
        "d2T": np.ascontiguousarray(np.asarray(D2, f32).T, np.float16),
        "b1": np.asarray(inputs["b1"], f32).reshape(H, 1),
        "b2": np.asarray(inputs["b2"], f32).reshape(H, 1),
        "db1": np.asarray(inputs["db1"], f32).reshape(H4, 1),
        "b3row": np.asarray(inputs["b3"], f32).reshape(1, H),
        "db2row": np.asarray(inputs["db2"], f32).reshape(1, H),
        "g1row": np.asarray(inputs["g1"], f32).reshape(1, H),
        "beta1row": np.asarray(inputs["beta1"], f32).reshape(1, H),
        "g2row": np.asarray(inputs["g2"], f32).reshape(1, H),
        "beta2row": np.asarray(inputs["beta2"], f32).reshape(1, H),
    }
    in_maps = []
    npc = nodes_per_core
    for c in range(n_cores):
        m = dict(shared)
        m["h_e"] = np.ascontiguousarray(he[c * npc * K:(c + 1) * npc * K])
        m["h_v"] = np.ascontiguousarray(hv[c * npc:(c + 1) * npc])
        m["mask_attend"] = np.ascontiguousarray(
            ma[c * npc:(c + 1) * npc]).reshape(npc // TPT, TPT * K)
        m["mask_v"] = np.ascontiguousarray(
            mv[c * npc:(c + 1) * npc]).reshape(128, npc // 128)
        in_maps.append(m)
    return in_maps


_NC_CACHE = {}


def kernel(**inputs) -> np.ndarray:
    nodes = B * N // N_CORES
    if nodes not in _NC_CACHE:
        _NC_CACHE[nodes] = build_bass(nodes)
    nc = _NC_CACHE[nodes]
    in_maps = make_in_maps(inputs, nodes, N_CORES)
    res = run_bass_kernel_spmd(nc, in_maps, core_ids=list(range(N_CORES)))
    outs = [r["out"] for r in res.results]           # each [H, nodes]
    full = np.concatenate(outs, axis=1)              # [H, B*N]
    return np.ascontiguousarray(full.T).reshape(B, N, H)
